# revision 17
# baseline (speedup 1.0000x reference)
"""Trainium2 Bass kernel for nn_Dota2Eq3Embed (2-tower equivariant set-net).

Math restructure (vs the reference einsum chain):
  e[n,i,:] = relu(embed[x[n,t,i]])                 (5 team members, d=64)
  t_{ijk}  = e_i * e_j * e_k   (elementwise over d)
  The 8 equivariant pooled ops all factor through S = sum_i e_i:
    g0 = t_{ijk}, g1 = S*u_{jk}, g2 = S*u_{ik}, g3 = S*u_{ij},
    g4 = S^2*e_k, g5 = S^2*e_j, g6 = S^2*e_i, g7 = S^3   (u = pair products)
  eq3 out at (i,j,k): out[s,ijk] = sum_g sum_d C_g[d,s] * F_g[d, m_g(ijk)]
  -> PSUM-accumulated matmuls per sample. fp8 moving data streams at 2x the
     fp16 rate on the PE, so all broadcast groups stream fp8: g1/g3/g5/g4/g6/
     g7 as 8-pair-batched 1000-col matmuls over the tiny fp8 feature blocks
     (SU, B[5a+b]=S2e[b], S2e, S3) via stride-0 access patterns; g0 streams
     fp16 t3; g2 ((i,k)-split, needs 3 pattern dims) goes per-pair, woven
     between the big streams where it runs in the idle PE quadrant half.
  The mean over (i,j,k) commutes with the second-layer contraction and is
  fused INTO the layer-2 matmul: the fold matmul streams relu(h) (fp8) against
  wout with a stride-0 PSUM output AP, accumulating all 125 positions into 5
  psum columns (summed by a tiny DVE reduce). The big (N,5,5,5,128) tensors of
  the reference are never materialized, and no DVE fold tree is needed.

Scaling: fp8e4m3 is inf above 240 and subnormal below ~2^-6, so features are
pre-scaled by per-tensor powers-of-two (alpha, host-computed from the actual
embed table) and stationaries carry beta = sigma/alpha; the common factor
sigma is divided out in the relu activation's scale argument. All scales ship
as input tensors, nothing data-dependent is baked into the program.

Sharding: pure data parallelism over the batch (2048 -> 8 x 256). The two
towers of one sample ride in SBUF partition halves 0-63 / 64-127 (d=64), so
every DVE op runs at full 128-partition width, and tower1/tower2 matmuls sit
in opposite PE quadrant halves where their streams overlap.
"""

import os
import sys
import dataclasses

import numpy as np

try:
    import concourse.bass as bass  # noqa: F401
except Exception:  # pragma: no cover - fresh grading container
    for _p in ("/opt/trn_rl_repo", "/root/.axon_site/_ro/trn_rl_repo"):
        if os.path.isdir(_p) and _p not in sys.path:
            sys.path.insert(0, _p)
    import concourse.bass as bass

import concourse.mybir as mybir
from concourse import bacc, tile
from concourse.bass_utils import run_bass_kernel_spmd

F32 = mybir.dt.float32
F16 = mybir.dt.float16
FP8 = mybir.dt.float8e4
I32 = mybir.dt.int32
ALU = mybir.AluOpType
ACTF = mybir.ActivationFunctionType
AXIS = mybir.AxisListType
DRM = mybir.MatmulPerfMode.DoubleRow
NP8 = mybir.dt.np(FP8)

N_CORES = 8
BATCH = 2048
N_LOC = BATCH // N_CORES          # 256 samples per core
TEAM = 5
D = 64                            # embed dim
HID = 128
OUT_DIM = 128
NEMBED = 128
NPAIR = N_LOC
WPAIRS = 8                        # pairs per psum window
NWIN = NPAIR // WPAIRS
FB = 60                           # fp8 feature cols per pair: SU 25 | B 25 | m2d 5 | pad 5
SIGMA = 256.0                     # common pre-activation scale


def _cap(tile_handle, plo, phi, coloff, pairs):
    """AP over tile partitions [plo:phi) with custom free-dim [step,count] list."""
    sl = tile_handle[plo:phi, 0:1]
    return dataclasses.replace(
        sl, offset=sl.offset + coloff, ap=[list(sl.ap[0])] + [list(p) for p in pairs]
    )


def build_nc():
    nc = bacc.Bacc(None, target_bir_lowering=False)

    x_d = nc.dram_tensor("x", [1, N_LOC * 2 * TEAM], I32, kind="ExternalInput")
    emb_d = nc.dram_tensor("embed", [NEMBED, D], F32, kind="ExternalInput")
    cg0_d = nc.dram_tensor("cg0", [128, HID], F32, kind="ExternalInput")
    c8s_d = nc.dram_tensor("c8s", [128, 7 * HID], F32, kind="ExternalInput")
    wo8_d = nc.dram_tensor("wo8", [HID, 2 * OUT_DIM], F32, kind="ExternalInput")
    b1ra_d = nc.dram_tensor("b1ra", [HID, 1], F32, kind="ExternalInput")
    b2ra_d = nc.dram_tensor("b2ra", [HID, 1], F32, kind="ExternalInput")
    bout1_d = nc.dram_tensor("bout1", [OUT_DIM, 1], F32, kind="ExternalInput")
    bout2_d = nc.dram_tensor("bout2", [OUT_DIM, 1], F32, kind="ExternalInput")
    scv_d = nc.dram_tensor("scv", [128, 8], F32, kind="ExternalInput")
    fcwhi_d = nc.dram_tensor("fcwhi", [OUT_DIM, 2], F32, kind="ExternalInput")
    fcwlo_d = nc.dram_tensor("fcwlo", [OUT_DIM, 2], F32, kind="ExternalInput")
    fcb_d = nc.dram_tensor("fcb", [1, 2], F32, kind="ExternalInput")
    ones_d = nc.dram_tensor("ones", [1, 128], F32, kind="ExternalInput")
    iota_d = nc.dram_tensor("iota", [128, 1], F32, kind="ExternalInput")
    out_d = nc.dram_tensor("out", [N_LOC, 2], F32, kind="ExternalOutput")

    NX = N_LOC * 2 * TEAM  # 2560 one-hot columns, col = (n*2+t)*5 + i

    with tile.TileContext(nc) as tc:
        with (
            nc.allow_low_precision(reason="fp8 feature pipeline, f32 psum accum"),
            tc.tile_pool(name="const", bufs=1) as cp,
            tc.tile_pool(name="feat", bufs=1) as fp,
            tc.tile_pool(name="relu", bufs=8) as rp,
            tc.tile_pool(name="eqps", bufs=3, space="PSUM") as pp,
            tc.tile_pool(name="miscps", bufs=2, space="PSUM") as pq,
        ):
            # ---- params -> SBUF ----
            emb_raw = cp.tile([NEMBED, D], F32)
            nc.sync.dma_start(emb_raw[:, :], emb_d[:, :])
            rel_emb = cp.tile([NEMBED, D], F32)
            nc.vector.tensor_scalar_max(rel_emb[:, :], emb_raw[:, :], 0.0)

            cg0f = cp.tile([128, HID], F32)
            nc.sync.dma_start(cg0f[:, :], cg0_d[:, :])
            cg0 = cp.tile([128, HID], F16)
            nc.vector.tensor_copy(cg0[:, :], cg0f[:, :])
            c8sf = cp.tile([128, 7 * HID], F32)
            nc.sync.dma_start(c8sf[:, :], c8s_d[:, :])
            c8s = cp.tile([128, 7 * HID], F16)
            nc.vector.tensor_copy(c8s[:, :], c8sf[:, :])
            wo8f = cp.tile([HID, 2 * OUT_DIM], F32)
            nc.sync.dma_start(wo8f[:, :], wo8_d[:, :])
            wo8 = cp.tile([HID, 2 * OUT_DIM], F16)
            nc.vector.tensor_copy(wo8[:, :], wo8f[:, :])

            b1ra = cp.tile([HID, 1], F32)
            nc.sync.dma_start(b1ra[:, :], b1ra_d[:, :])
            b2ra = cp.tile([HID, 1], F32)
            nc.sync.dma_start(b2ra[:, :], b2ra_d[:, :])
            bout1 = cp.tile([OUT_DIM, 1], F32)
            nc.sync.dma_start(bout1[:, :], bout1_d[:, :])
            bout2 = cp.tile([OUT_DIM, 1], F32)
            nc.sync.dma_start(bout2[:, :], bout2_d[:, :])
            scv = cp.tile([128, 8], F32)
            nc.sync.dma_start(scv[:, :], scv_d[:, :])
            fcwhi = cp.tile([OUT_DIM, 2], F32)
            nc.sync.dma_start(fcwhi[:, :], fcwhi_d[:, :])
            fcwlo = cp.tile([OUT_DIM, 2], F32)
            nc.sync.dma_start(fcwlo[:, :], fcwlo_d[:, :])
            fcb = cp.tile([1, 2], F32)
            nc.sync.dma_start(fcb[:, :], fcb_d[:, :])
            ones = cp.tile([1, 128], F32)
            nc.sync.dma_start(ones[:, :], ones_d[:, :])
            iota = cp.tile([128, 1], F32)
            nc.sync.dma_start(iota[:, :], iota_d[:, :])

            # ---- one-hot of x ----
            xsb = cp.tile([1, NX], I32)
            nc.sync.dma_start(xsb[:, :], x_d[:, :])
            xf = cp.tile([1, NX], F16)
            nc.scalar.copy(xf[:, :], xsb[:, :])
            ones16 = cp.tile([1, 128], F16)
            nc.vector.tensor_copy(ones16[:, :], ones[:, :])

            onehot = cp.tile([128, NX], F32)
            for c in range(NX // 512):
                pidx = pq.tile([128, 512], F32, tag="ps")
                nc.tensor.matmul(
                    pidx[:, :], ones16[:, :], xf[:, c * 512:(c + 1) * 512],
                    start=True, stop=True,
                )
                nc.vector.tensor_scalar(
                    onehot[:, c * 512:(c + 1) * 512], pidx[:, :],
                    iota[:, 0:1], None, op0=ALU.is_equal,
                )

            # ---- gather: e_sb[0:64] = even st (tower1), [64:128] = odd (tower2)
            e_sb = cp.tile([128, NPAIR * TEAM], F32)
            GCHUNKS = [(0, 16), (16, 48), (64, 64), (128, 64), (192, 64)]
            for cstart, cn in GCHUNKS:
                pg = pq.tile([128, 64 * TEAM], F32, tag="ps")
                for h in range(2):
                    rhs = _cap(onehot, 0, 128, cstart * 10 + h * TEAM,
                               [[10, cn], [1, TEAM]])
                    nc.tensor.matmul(
                        pg[h * 64:(h + 1) * 64, 0:cn * TEAM], rel_emb[:, :],
                        rhs, start=True, stop=True, tile_position=(0, h * 64),
                    )
                nc.scalar.copy(
                    e_sb[:, cstart * TEAM:(cstart + cn) * TEAM],
                    pg[:, 0:cn * TEAM])

            feats = []
            BLOCKS = [(0, 8), (8, 8), (16, 16), (32, 32), (64, 64),
                      (128, 64), (192, 64)]

            def emit_feats(b):
                bstart, nb = BLOCKS[b]
                ecol = bstart * TEAM
                # centered basis: dl = e - m (m = S/5); features dl^3,
                # m*dl*dl, m^2*dl, m^3 are all ~emax^3 scale, so the huge
                # cancellations between pooled ops happen exactly in f32
                # instead of across quantized fp8 values.
                S = fp.tile([128, nb], F32, tag=f"S{b}")
                nc.vector.tensor_reduce(
                    S[:, :],
                    _cap(e_sb, 0, 128, ecol, [[5, nb], [1, 5]]),
                    axis=AXIS.X, op=ALU.add,
                )
                mneg = fp.tile([128, nb], F32, tag=f"mneg{b}")
                nc.vector.tensor_scalar_mul(mneg[:, :], S[:, :], -0.2)
                dl = fp.tile([128, nb * TEAM], F32, tag=f"dl{b}")
                nc.vector.tensor_add(
                    dl[:, :],
                    _cap(e_sb, 0, 128, ecol, [[5, nb], [1, 5]]),
                    _cap(mneg, 0, 128, 0, [[1, nb], [0, 5]]),
                )
                u = fp.tile([128, nb * 25], F32, tag=f"u{b}")
                nc.vector.tensor_mul(
                    u[:, :],
                    _cap(dl, 0, 128, 0, [[5, nb], [1, 5], [0, 5]]),
                    _cap(dl, 0, 128, 0, [[5, nb], [0, 5], [1, 5]]),
                )
                t3 = fp.tile([128, nb * 125], F16, tag=f"t3{b}")
                nc.vector.tensor_mul(
                    t3[:, :],
                    _cap(dl, 0, 128, 0, [[5, nb], [1, 5], [0, 25]]),
                    _cap(u, 0, 128, 0, [[25, nb], [0, 5], [1, 25]]),
                )
                S2 = fp.tile([128, nb], F32, tag=f"S2{b}")
                nc.vector.tensor_mul(S2[:, :], S[:, :], S[:, :])
                Sa = fp.tile([128, nb], F32, tag=f"Sa{b}")
                nc.vector.tensor_scalar_mul(Sa[:, :], S[:, :], scv[:, 0:1])
                S3a = fp.tile([128, nb], F32, tag=f"S3a{b}")
                nc.vector.tensor_mul(S3a[:, :], S2[:, :], Sa[:, :])
                # m3 stays fp16: its contribution is a large per-pair offset,
                # fp8 on it costs 3e-2 of final relative error
                m3 = fp.tile([128, nb], F16, tag=f"m3{b}")
                nc.vector.tensor_scalar_mul(m3[:, :], S3a[:, :], scv[:, 2:3])
                m2d = fp.tile([128, nb * TEAM], F32, tag=f"m2d{b}")
                nc.vector.tensor_mul(
                    m2d[:, :],
                    _cap(dl, 0, 128, 0, [[5, nb], [1, 5]]),
                    _cap(S2, 0, 128, 0, [[1, nb], [0, 5]]),
                )

                # fp8 block per pair: [SU 25 | B 25 | m2d8 5 | pad 5]
                fb = fp.tile([128, nb * FB], FP8, tag=f"fb{b}")
                # SU8 = (m*u)*a = u * (S*0.2a): cols 0-25
                nc.vector.tensor_mul(
                    _cap(fb, 0, 128, 0, [[FB, nb], [1, 25]]),
                    _cap(u, 0, 128, 0, [[25, nb], [1, 25]]),
                    _cap(Sa, 0, 128, 0, [[1, nb], [0, 25]]),
                )
                # B[5a+b] = m2d[b]*0.04a: cols 25-50
                nc.vector.tensor_scalar_mul(
                    _cap(fb, 0, 128, 25, [[FB, nb], [1, 25]]),
                    _cap(m2d, 0, 128, 0, [[5, nb], [0, 5], [1, 5]]),
                    scv[:, 1:2],
                )
                # m2d8: cols 50-55
                nc.vector.tensor_scalar_mul(
                    _cap(fb, 0, 128, 50, [[FB, nb], [1, 5]]),
                    _cap(m2d, 0, 128, 0, [[5, nb], [1, 5]]),
                    scv[:, 1:2],
                )
                feats.append((t3, fb, m3, bstart))

            def blk_of(pair):
                for (t3, fb, m3, bstart), (bs, nb) in zip(feats, BLOCKS):
                    if bs <= pair < bs + nb:
                        return t3, fb, m3, bs
                raise AssertionError

            pre2 = cp.tile([128, 2 * NPAIR], F32)

            def emit_window(w, prev):
                p0 = w * WPAIRS
                t3, fb, m3, bs = blk_of(p0)
                po = p0 - bs
                pt = []
                for t in range(2):
                    ptile = pp.tile([128, 1024], F32, tag="pt", name=f"pt{t}_{w}")
                    pt.append(ptile)
                # big streams: 8-pair-batched matmuls, tower-alternating so
                # consecutive instructions stream in opposite PE quadrant
                # halves (they overlap; LDWEIGHTS hides under the streams).
                # g2 ((i,k)-split) cannot batch pairs within 3 AP dims, so its
                # per-pair 125-col matmuls are woven between the big streams
                # where they run in the idle quadrant.
                BIGS = [
                    (None, 0, "t3"),            # g0: fp16 t3 (dl^3)
                    (0 * HID, 0, "fb"),         # g1: SU [[0,5],[1,25]]
                    (1 * HID, 0, "fb"),         # g3: SU [[1,25],[0,5]]
                    (2 * HID, 25, "fb"),        # g5: B  [[1,25],[0,5]]
                    (3 * HID, 50, "fb"),        # g4: m2d8 [[0,25],[1,5]]
                    (4 * HID, 50, "fb"),        # g6: m2d8 [[1,5],[0,25]]
                    (5 * HID, 0, "m3"),         # g7: fp16 m3 [[1,4],[0,125]]
                ]
                # psum matmul writes must stay inside one 512-col bank:
                # bank b holds pairs 4b..4b+3 at cols b*512 + q*125.
                BIGPATS = [
                    [[125, 4], [1, 125]],
                    [[FB, 4], [0, 5], [1, 25]],
                    [[FB, 4], [1, 25], [0, 5]],
                    [[FB, 4], [1, 25], [0, 5]],
                    [[FB, 4], [0, 25], [1, 5]],
                    [[FB, 4], [1, 5], [0, 25]],
                    [[1, 4], [0, 125]],
                ]
                def emit_small(t, p, stop=False):
                    b, q = divmod(p, 4)
                    nc.tensor.matmul(
                        pt[t][:, b * 512 + q * 125:b * 512 + q * 125 + 125],
                        c8s[t * 64:(t + 1) * 64, 6 * HID:7 * HID],
                        _cap(fb, t * 64, (t + 1) * 64, (po + p) * FB,
                             [[5, 5], [0, 5], [1, 5]]),
                        start=False, stop=stop, tile_position=(t * 64, 0),
                        skip_group_check=True,
                    )

                for k, (coff, fboff, kind) in enumerate(BIGS):
                    for b in range(2):
                        for t in range(2):
                            if kind == "t3":
                                lhsT = cg0[t * 64:(t + 1) * 64, :]
                                rhs = _cap(t3, t * 64, (t + 1) * 64,
                                           (po + b * 4) * 125, BIGPATS[k])
                            elif kind == "m3":
                                lhsT = c8s[t * 64:(t + 1) * 64, coff:coff + HID]
                                rhs = _cap(m3, t * 64, (t + 1) * 64,
                                           po + b * 4, BIGPATS[k])
                            else:
                                lhsT = c8s[t * 64:(t + 1) * 64, coff:coff + HID]
                                rhs = _cap(fb, t * 64, (t + 1) * 64,
                                           (po + b * 4) * FB + fboff,
                                           BIGPATS[k])
                            nc.tensor.matmul(
                                pt[t][:, b * 512:b * 512 + 500], lhsT, rhs,
                                start=(k == 0), stop=False,
                                tile_position=(t * 64, 0),
                                skip_group_check=True,
                            )
                # g2 per-pair matmuls: one weight load per tower, then 8
                # back-to-back 125-col streams
                for t in range(2):
                    for p in range(WPAIRS):
                        emit_small(t, p, stop=(p == WPAIRS - 1))

                # fold for the PREVIOUS window (its relu has completed by now,
                # so the PE never stalls waiting on the Scalar engine)
                if prev is not None:
                    emit_fold(*prev)

                # relu -> fp8 ra (scaled); fold consumes it next window
                ras = []
                for t in range(2):
                    ra = rp.tile([128, 1024], FP8, tag=f"ra{t}", name=f"ra{t}_{w}")
                    nc.scalar.activation(
                        _cap(ra, 0, 128, 0, [[500, 2], [1, 500]]),
                        _cap(pt[t], 0, 128, 0, [[512, 2], [1, 500]]),
                        ACTF.Relu,
                        bias=(b1ra if t == 0 else b2ra)[:, 0:1],
                        scale=scv[:, 3:4])
                    ras.append(ra)
                return (w, ras)

            def emit_fold(w, ras):
                fps = pq.tile([128, 512], F32, tag="ps", name=f"fps_{w}")
                for t in range(2):
                    nc.tensor.matmul(
                        _cap(fps, 0, 128, t * 200, [[25, WPAIRS], [1, 25]]),
                        wo8[:, t * 128:(t + 1) * 128],
                        _cap(ras[t], 0, 128, 0, [[125, WPAIRS], [1, 25]]),
                        start=True, stop=False, skip_group_check=True,
                    )
                    for p in range(WPAIRS):
                        nc.tensor.matmul(
                            _cap(fps, 0, 128, t * 200 + p * 25,
                                 [[0, 4], [1, 25]]),
                            wo8[:, t * 128:(t + 1) * 128],
                            _cap(ras[t], 0, 128, p * 125 + 25, [[1, 100]]),
                            start=False, stop=(p == WPAIRS - 1),
                            skip_group_check=True,
                        )
                nc.vector.tensor_reduce(
                    _cap(pre2, 0, 128, w * WPAIRS, [[NPAIR, 2], [1, WPAIRS]]),
                    _cap(fps, 0, 128, 0, [[200, 2], [25, WPAIRS], [1, 25]]),
                    axis=AXIS.X, op=ALU.add,
                )

            with tc.high_priority():
                for b in range(len(BLOCKS)):
                    emit_feats(b)
            prev = None
            for w in range(NWIN):
                prev = emit_window(w, prev)
            emit_fold(*prev)

            # ---- z = relu(pre2 * sc_z + bout) ----
            z1 = cp.tile([OUT_DIM, NPAIR], F32)
            z2 = cp.tile([OUT_DIM, NPAIR], F32)
            nc.scalar.activation(z1[:, :], pre2[:, 0:NPAIR], ACTF.Relu,
                                 bias=bout1[:, 0:1], scale=scv[:, 4:5])
            nc.scalar.activation(z2[:, :], pre2[:, NPAIR:2 * NPAIR], ACTF.Relu,
                                 bias=bout2[:, 0:1], scale=scv[:, 4:5])

            # ---- final fc: out = z @ fcw + fcb ----
            outsb = cp.tile([128, 2 * (N_LOC // 128)], F32)
            pfc = pq.tile([128, 512], F32, tag="ps")
            for ch in range(N_LOC // 128):
                sl = pfc[:, ch * 8:ch * 8 + 2]
                nc.tensor.matmul(sl, z1[:, ch * 128:(ch + 1) * 128],
                                 fcwhi[:, :], start=True, stop=False)
                nc.tensor.matmul(sl, z2[:, ch * 128:(ch + 1) * 128],
                                 fcwlo[:, :], start=False, stop=False)
                nc.tensor.matmul(sl, ones[:, :], fcb[:, :],
                                 start=False, stop=True)
            nc.vector.tensor_copy(
                _cap(outsb, 0, 128, 0, [[2, 2], [1, 2]]),
                _cap(pfc, 0, 128, 0, [[8, 2], [1, 2]]),
            )
            for ch in range(N_LOC // 128):
                nc.sync.dma_start(out_d[ch * 128:(ch + 1) * 128, :],
                                  outsb[:, ch * 2:(ch + 1) * 2])

    nc.compile()
    return nc


def _pow2_below(x):
    return float(2.0 ** np.floor(np.log2(max(x, 1e-30))))


def make_in_maps(inputs):
    x = np.ascontiguousarray(np.asarray(inputs["x"], dtype=np.int32))
    embed = np.asarray(inputs["embed"], dtype=np.float32)
    coefs1 = np.asarray(inputs["coefs1"], dtype=np.float32)
    coefs2 = np.asarray(inputs["coefs2"], dtype=np.float32)
    wout1 = np.asarray(inputs["wout1"], np.float32)
    wout2 = np.asarray(inputs["wout2"], np.float32)

    e = np.maximum(embed, 0.0)
    emax = float(e.max()) + 1e-12
    # centered-basis features are all bounded by ~emax^3 -> one fp8 scale
    alpha = _pow2_below(200.0 / (emax ** 3))
    ra_scale = 32.0
    ws = 1.0

    def stk2(c1, c2, beta):
        m = np.zeros((128, HID), np.float32)
        m[0:64] = c1 * beta
        m[64:128] = c2 * beta
        return m

    def cs_transform(coefs):
        C = [coefs[:, :, g] for g in range(8)]
        return [
            C[0], C[0] + 5 * C[1], C[0] + 5 * C[2], C[0] + 5 * C[3],
            C[0] + 5 * C[1] + 5 * C[2] + 25 * C[4],
            C[0] + 5 * C[1] + 5 * C[3] + 25 * C[5],
            C[0] + 5 * C[2] + 5 * C[3] + 25 * C[6],
            C[0] + 5 * (C[1] + C[2] + C[3])
            + 25 * (C[4] + C[5] + C[6]) + 125 * C[7],
        ]

    Cp1 = cs_transform(coefs1)
    Cp2 = cs_transform(coefs2)

    cg0 = stk2(Cp1[0], Cp2[0], SIGMA)

    # fp16 stationaries, col blocks in stream order [g1 g3 g5 g4 g6 g7 g2]
    c8s = np.zeros((128, 7 * HID), np.float32)
    for bi, g in enumerate([1, 3, 5, 4, 6, 7, 2]):
        c8s[:, bi * HID:(bi + 1) * HID] = stk2(Cp1[g], Cp2[g], SIGMA / alpha)

    wo8 = np.zeros((HID, 2 * OUT_DIM), np.float32)
    wo8[:, 0:OUT_DIM] = wout1 * ws
    wo8[:, OUT_DIM:] = wout2 * ws

    scv = np.zeros((128, 8), np.float32)
    scv[:, 0] = 0.2 * alpha           # Sa = S*scv0 = m*alpha
    scv[:, 1] = 0.04 * alpha          # m2d8 = (dl*S2)*scv1 = m^2*dl*alpha
    scv[:, 2] = 0.04                  # m3 = S3a*scv2 = m^3*alpha (S3a = 25 m^3 alpha)
    scv[:, 3] = ra_scale / SIGMA
    scv[:, 4] = 1.0 / (125.0 * ra_scale * ws)

    common = {
        "embed": np.ascontiguousarray(embed),
        "cg0": cg0,
        "c8s": c8s,
        "wo8": wo8,
        "b1ra": (np.asarray(inputs["bias1"], np.float32) * ra_scale
                 ).reshape(HID, 1).copy(),
        "b2ra": (np.asarray(inputs["bias2"], np.float32) * ra_scale
                 ).reshape(HID, 1).copy(),
        "bout1": np.asarray(inputs["bout1"], np.float32).reshape(OUT_DIM, 1).copy(),
        "bout2": np.asarray(inputs["bout2"], np.float32).reshape(OUT_DIM, 1).copy(),
        "scv": scv,
        "fcwhi": np.ascontiguousarray(np.asarray(inputs["fcw"], np.float32)[0:OUT_DIM]),
        "fcwlo": np.ascontiguousarray(np.asarray(inputs["fcw"], np.float32)[OUT_DIM:]),
        "fcb": np.asarray(inputs["fcb"], np.float32).reshape(1, 2).copy(),
        "ones": np.ones((1, 128), np.float32),
        "iota": np.arange(128, dtype=np.float32).reshape(128, 1).copy(),
    }
    in_maps = []
    for c in range(N_CORES):
        m = dict(common)
        m["x"] = x[c * N_LOC:(c + 1) * N_LOC].reshape(1, N_LOC * 2 * TEAM).copy()
        in_maps.append(m)
    return in_maps


_NC = None


def kernel(**inputs):
    global _NC
    if _NC is None:
        _NC = build_nc()
    in_maps = make_in_maps(inputs)
    res = run_bass_kernel_spmd(_NC, in_maps, core_ids=list(range(N_CORES)))
    return np.concatenate([r["out"] for r in res.results], axis=0)


if __name__ == "__main__":
    nc = build_nc()
    print("built ok")


# revision 21
# speedup vs baseline: 1.0469x; 1.0469x over previous
"""Trainium2 Bass kernel for nn_Dota2Eq3Embed (2-tower equivariant set-net).

Math restructure (vs the reference einsum chain):
  e[n,i,:] = relu(embed[x[n,t,i]])                 (5 team members, d=64)
  t_{ijk}  = e_i * e_j * e_k   (elementwise over d)
  The 8 equivariant pooled ops all factor through S = sum_i e_i:
    g0 = t_{ijk}, g1 = S*u_{jk}, g2 = S*u_{ik}, g3 = S*u_{ij},
    g4 = S^2*e_k, g5 = S^2*e_j, g6 = S^2*e_i, g7 = S^3   (u = pair products)
  eq3 out at (i,j,k): out[s,ijk] = sum_g sum_d C_g[d,s] * F_g[d, m_g(ijk)]
  -> PSUM-accumulated matmuls per sample. fp8 moving data streams at 2x the
     fp16 rate on the PE, so all broadcast groups stream fp8: g1/g3/g5/g4/g6/
     g7 as 8-pair-batched 1000-col matmuls over the tiny fp8 feature blocks
     (SU, B[5a+b]=S2e[b], S2e, S3) via stride-0 access patterns; g0 streams
     fp16 t3; g2 ((i,k)-split, needs 3 pattern dims) goes per-pair, woven
     between the big streams where it runs in the idle PE quadrant half.
  The mean over (i,j,k) commutes with the second-layer contraction and is
  fused INTO the layer-2 matmul: the fold matmul streams relu(h) (fp8) against
  wout with a stride-0 PSUM output AP, accumulating all 125 positions into 5
  psum columns (summed by a tiny DVE reduce). The big (N,5,5,5,128) tensors of
  the reference are never materialized, and no DVE fold tree is needed.

Scaling: fp8e4m3 is inf above 240 and subnormal below ~2^-6, so features are
pre-scaled by per-tensor powers-of-two (alpha, host-computed from the actual
embed table) and stationaries carry beta = sigma/alpha; the common factor
sigma is divided out in the relu activation's scale argument. All scales ship
as input tensors, nothing data-dependent is baked into the program.

Sharding: pure data parallelism over the batch (2048 -> 8 x 256). The two
towers of one sample ride in SBUF partition halves 0-63 / 64-127 (d=64), so
every DVE op runs at full 128-partition width, and tower1/tower2 matmuls sit
in opposite PE quadrant halves where their streams overlap.
"""

import os
import sys
import dataclasses

import numpy as np

try:
    import concourse.bass as bass  # noqa: F401
except Exception:  # pragma: no cover - fresh grading container
    for _p in ("/opt/trn_rl_repo", "/root/.axon_site/_ro/trn_rl_repo"):
        if os.path.isdir(_p) and _p not in sys.path:
            sys.path.insert(0, _p)
    import concourse.bass as bass

import concourse.mybir as mybir
from concourse import bacc, tile
from concourse.bass_utils import run_bass_kernel_spmd

F32 = mybir.dt.float32
F16 = mybir.dt.float16
FP8 = mybir.dt.float8e4
I32 = mybir.dt.int32
ALU = mybir.AluOpType
ACTF = mybir.ActivationFunctionType
AXIS = mybir.AxisListType
DRM = mybir.MatmulPerfMode.DoubleRow
NP8 = mybir.dt.np(FP8)

N_CORES = 8
BATCH = 2048
N_LOC = BATCH // N_CORES          # 256 samples per core
TEAM = 5
D = 64                            # embed dim
HID = 128
OUT_DIM = 128
NEMBED = 128
NPAIR = N_LOC
WPAIRS = 8                        # pairs per psum window
NWIN = NPAIR // WPAIRS
FB = 60                           # fp8 feature cols per pair: SU 25 | B 25 | m2d 5 | pad 5
SIGMA = 256.0                     # common pre-activation scale


def _cap(tile_handle, plo, phi, coloff, pairs):
    """AP over tile partitions [plo:phi) with custom free-dim [step,count] list."""
    sl = tile_handle[plo:phi, 0:1]
    return dataclasses.replace(
        sl, offset=sl.offset + coloff, ap=[list(sl.ap[0])] + [list(p) for p in pairs]
    )


def build_nc():
    nc = bacc.Bacc(None, target_bir_lowering=False)

    x_d = nc.dram_tensor("x", [1, N_LOC * 2 * TEAM], I32, kind="ExternalInput")
    emb_d = nc.dram_tensor("embed", [NEMBED, D], F32, kind="ExternalInput")
    c8s_d = nc.dram_tensor("c8s", [128, 6 * HID], F32, kind="ExternalInput")
    c02_d = nc.dram_tensor("c02", [128, 2 * HID], F32, kind="ExternalInput")
    wo8_d = nc.dram_tensor("wo8", [HID, 2 * OUT_DIM], F32, kind="ExternalInput")
    b1ra_d = nc.dram_tensor("b1ra", [HID, 1], F32, kind="ExternalInput")
    b2ra_d = nc.dram_tensor("b2ra", [HID, 1], F32, kind="ExternalInput")
    bout1_d = nc.dram_tensor("bout1", [OUT_DIM, 1], F32, kind="ExternalInput")
    bout2_d = nc.dram_tensor("bout2", [OUT_DIM, 1], F32, kind="ExternalInput")
    scv_d = nc.dram_tensor("scv", [128, 8], F32, kind="ExternalInput")
    fcwhi_d = nc.dram_tensor("fcwhi", [OUT_DIM, 2], F32, kind="ExternalInput")
    fcwlo_d = nc.dram_tensor("fcwlo", [OUT_DIM, 2], F32, kind="ExternalInput")
    fcb_d = nc.dram_tensor("fcb", [1, 2], F32, kind="ExternalInput")
    ones_d = nc.dram_tensor("ones", [1, 128], F32, kind="ExternalInput")
    iota_d = nc.dram_tensor("iota", [128, 1], F32, kind="ExternalInput")
    out_d = nc.dram_tensor("out", [N_LOC, 2], F32, kind="ExternalOutput")

    NX = N_LOC * 2 * TEAM  # 2560 one-hot columns, col = (n*2+t)*5 + i

    with tile.TileContext(nc) as tc:
        with (
            nc.allow_low_precision(reason="fp8 feature pipeline, f32 psum accum"),
            tc.tile_pool(name="const", bufs=1) as cp,
            tc.tile_pool(name="feat", bufs=1) as fp,
            tc.tile_pool(name="relu", bufs=8) as rp,
            tc.tile_pool(name="eqps", bufs=3, space="PSUM") as pp,
            tc.tile_pool(name="miscps", bufs=2, space="PSUM") as pq,
        ):
            # ---- params -> SBUF ----
            emb_raw = cp.tile([NEMBED, D], F32)
            nc.sync.dma_start(emb_raw[:, :], emb_d[:, :])
            rel_emb = cp.tile([NEMBED, D], F16)
            nc.vector.tensor_scalar_max(rel_emb[:, :], emb_raw[:, :], 0.0)

            c8sf = cp.tile([128, 6 * HID], F32)
            nc.sync.dma_start(c8sf[:, :], c8s_d[:, :])
            c8s = cp.tile([128, 6 * HID], F16)
            nc.vector.tensor_copy(c8s[:, :], c8sf[:, :])
            c02f = cp.tile([128, 2 * HID], F32)
            nc.sync.dma_start(c02f[:, :], c02_d[:, :])
            c02 = cp.tile([128, 2 * HID], FP8)
            nc.vector.tensor_copy(c02[:, :], c02f[:, :])
            wo8f = cp.tile([HID, 2 * OUT_DIM], F32)
            nc.sync.dma_start(wo8f[:, :], wo8_d[:, :])
            wo8 = cp.tile([HID, 2 * OUT_DIM], F16)
            nc.vector.tensor_copy(wo8[:, :], wo8f[:, :])

            b1ra = cp.tile([HID, 1], F32)
            nc.sync.dma_start(b1ra[:, :], b1ra_d[:, :])
            b2ra = cp.tile([HID, 1], F32)
            nc.sync.dma_start(b2ra[:, :], b2ra_d[:, :])
            bout1 = cp.tile([OUT_DIM, 1], F32)
            nc.sync.dma_start(bout1[:, :], bout1_d[:, :])
            bout2 = cp.tile([OUT_DIM, 1], F32)
            nc.sync.dma_start(bout2[:, :], bout2_d[:, :])
            scv = cp.tile([128, 8], F32)
            nc.sync.dma_start(scv[:, :], scv_d[:, :])
            fcwhi = cp.tile([OUT_DIM, 2], F32)
            nc.sync.dma_start(fcwhi[:, :], fcwhi_d[:, :])
            fcwlo = cp.tile([OUT_DIM, 2], F32)
            nc.sync.dma_start(fcwlo[:, :], fcwlo_d[:, :])
            fcb = cp.tile([1, 2], F32)
            nc.sync.dma_start(fcb[:, :], fcb_d[:, :])
            ones = cp.tile([1, 128], F32)
            nc.sync.dma_start(ones[:, :], ones_d[:, :])
            iota = cp.tile([128, 1], F32)
            nc.sync.dma_start(iota[:, :], iota_d[:, :])

            # ---- one-hot of x ----
            xsb = cp.tile([1, NX], I32)
            nc.sync.dma_start(xsb[:, :], x_d[:, :])
            xf = cp.tile([1, NX], F16)
            nc.scalar.copy(xf[:, :], xsb[:, :])
            ones16 = cp.tile([1, 128], F16)
            nc.vector.tensor_copy(ones16[:, :], ones[:, :])

            onehot = cp.tile([128, NX], F16)
            for c in range(NX // 512):
                pidx = pq.tile([128, 512], F32, tag="ps")
                nc.tensor.matmul(
                    pidx[:, :], ones16[:, :], xf[:, c * 512:(c + 1) * 512],
                    start=True, stop=True,
                )
                nc.vector.tensor_scalar(
                    onehot[:, c * 512:(c + 1) * 512], pidx[:, :],
                    iota[:, 0:1], None, op0=ALU.is_equal,
                )

            # ---- gather: e_sb[0:64] = even st (tower1), [64:128] = odd (tower2)
            e_sb = cp.tile([128, NPAIR * TEAM], F32)
            GCHUNKS = [(0, 16), (16, 48), (64, 64), (128, 64), (192, 64)]
            for cstart, cn in GCHUNKS:
                pg = pq.tile([128, 64 * TEAM], F32, tag="ps")
                for h in range(2):
                    rhs = _cap(onehot, 0, 128, cstart * 10 + h * TEAM,
                               [[10, cn], [1, TEAM]])
                    nc.tensor.matmul(
                        pg[h * 64:(h + 1) * 64, 0:cn * TEAM], rel_emb[:, :],
                        rhs, start=True, stop=True, tile_position=(0, h * 64),
                    )
                nc.scalar.copy(
                    e_sb[:, cstart * TEAM:(cstart + cn) * TEAM],
                    pg[:, 0:cn * TEAM])

            feats = []
            BLOCKS = [(0, 8), (8, 8), (16, 16), (32, 32), (64, 64),
                      (128, 64), (192, 64)]

            def emit_feats(b):
                bstart, nb = BLOCKS[b]
                ecol = bstart * TEAM
                # centered basis: dl = e - m (m = S/5); features dl^3,
                # m*dl*dl, m^2*dl, m^3 are all ~emax^3 scale, so the huge
                # cancellations between pooled ops happen exactly in f32
                # instead of across quantized fp8 values.
                S = fp.tile([128, nb], F32, tag=f"S{b}")
                nc.vector.tensor_reduce(
                    S[:, :],
                    _cap(e_sb, 0, 128, ecol, [[5, nb], [1, 5]]),
                    axis=AXIS.X, op=ALU.add,
                )
                mneg = fp.tile([128, nb], F32, tag=f"mneg{b}")
                nc.vector.tensor_scalar_mul(mneg[:, :], S[:, :], -0.2)
                dl = fp.tile([128, nb * TEAM], F32, tag=f"dl{b}")
                nc.vector.tensor_add(
                    dl[:, :],
                    _cap(e_sb, 0, 128, ecol, [[5, nb], [1, 5]]),
                    _cap(mneg, 0, 128, 0, [[1, nb], [0, 5]]),
                )
                u = fp.tile([128, nb * 25], F32, tag=f"u{b}")
                nc.vector.tensor_mul(
                    u[:, :],
                    _cap(dl, 0, 128, 0, [[5, nb], [1, 5], [0, 5]]),
                    _cap(dl, 0, 128, 0, [[5, nb], [0, 5], [1, 5]]),
                )
                # combined fp8 tile for the (g0,g2) DoubleRow stream:
                # per pair [t38 = dl^3 (125) | SUik[25i+5j+k] = SU8[ik] (125)]
                g02 = fp.tile([128, nb * 250], FP8, tag=f"g02{b}")
                nc.vector.tensor_mul(
                    _cap(g02, 0, 128, 0, [[250, nb], [1, 125]]),
                    _cap(dl, 0, 128, 0, [[5, nb], [1, 5], [0, 25]]),
                    _cap(u, 0, 128, 0, [[25, nb], [0, 5], [1, 25]]),
                )
                S2 = fp.tile([128, nb], F32, tag=f"S2{b}")
                nc.vector.tensor_mul(S2[:, :], S[:, :], S[:, :])
                Sa = fp.tile([128, nb], F32, tag=f"Sa{b}")
                nc.vector.tensor_scalar_mul(Sa[:, :], S[:, :], scv[:, 0:1])
                S3a = fp.tile([128, nb], F32, tag=f"S3a{b}")
                nc.vector.tensor_mul(S3a[:, :], S2[:, :], Sa[:, :])
                # m3 stays fp16: its contribution is a large per-pair offset,
                # fp8 on it costs 3e-2 of final relative error
                m3 = fp.tile([128, nb], F16, tag=f"m3{b}")
                nc.vector.tensor_scalar_mul(m3[:, :], S3a[:, :], scv[:, 2:3])
                m2d = fp.tile([128, nb * TEAM], F32, tag=f"m2d{b}")
                nc.vector.tensor_mul(
                    m2d[:, :],
                    _cap(dl, 0, 128, 0, [[5, nb], [1, 5]]),
                    _cap(S2, 0, 128, 0, [[1, nb], [0, 5]]),
                )

                # fp8 block per pair: [SU 25 | B 25 | m2d8 5 | pad 5]
                fb = fp.tile([128, nb * FB], FP8, tag=f"fb{b}")
                # SU8 = (m*u)*a = u * (S*0.2a): cols 0-25
                nc.vector.tensor_mul(
                    _cap(fb, 0, 128, 0, [[FB, nb], [1, 25]]),
                    _cap(u, 0, 128, 0, [[25, nb], [1, 25]]),
                    _cap(Sa, 0, 128, 0, [[1, nb], [0, 25]]),
                )
                # B[5a+b] = m2d[b]*0.04a: cols 25-50
                nc.vector.tensor_scalar_mul(
                    _cap(fb, 0, 128, 25, [[FB, nb], [1, 25]]),
                    _cap(m2d, 0, 128, 0, [[5, nb], [0, 5], [1, 5]]),
                    scv[:, 1:2],
                )
                # m2d8: cols 50-55
                nc.vector.tensor_scalar_mul(
                    _cap(fb, 0, 128, 50, [[FB, nb], [1, 5]]),
                    _cap(m2d, 0, 128, 0, [[5, nb], [1, 5]]),
                    scv[:, 1:2],
                )
                # SUik expansion (ktile1 of the DR stream), one op per i
                for i in range(5):
                    nc.vector.tensor_copy(
                        _cap(g02, 0, 128, 125 + 25 * i, [[250, nb], [1, 25]]),
                        _cap(fb, 0, 128, 5 * i, [[FB, nb], [0, 5], [1, 5]]),
                    )
                feats.append((g02, fb, m3, bstart))

            def blk_of(pair):
                for (g02, fb, m3, bstart), (bs, nb) in zip(feats, BLOCKS):
                    if bs <= pair < bs + nb:
                        return g02, fb, m3, bs
                raise AssertionError

            pre2 = cp.tile([128, 2 * NPAIR], F32)

            def emit_window(w, prev):
                p0 = w * WPAIRS
                g02, fb, m3, bs = blk_of(p0)
                po = p0 - bs
                pt = []
                for t in range(2):
                    ptile = pp.tile([128, 1024], F32, tag="pt", name=f"pt{t}_{w}")
                    pt.append(ptile)
                # (g0,g2) fused DoubleRow stream first (start=True), then
                # the 6 broadcast groups as 8-pair-batched fp8 matmuls,
                # tower-alternating so consecutive instructions stream in
                # opposite PE quadrant halves.
                for b in range(2):
                    for t in range(2):
                        nc.tensor.matmul(
                            pt[t][:, b * 512:b * 512 + 500],
                            _cap(c02, t * 64, (t + 1) * 64, 0,
                                 [[128, 2], [1, 128]]),
                            _cap(g02, t * 64, (t + 1) * 64, (po + b * 4) * 250,
                                 [[125, 2], [250, 4], [1, 125]]),
                            start=True, stop=False, perf_mode=DRM,
                            tile_position=(t * 64, 0), skip_group_check=True,
                        )
                BIGS = [
                    (0 * HID, 0),         # g1: SU [[0,5],[1,25]]
                    (1 * HID, 0),         # g3: SU [[1,25],[0,5]]
                    (2 * HID, 25),        # g5: B  [[1,25],[0,5]]
                    (3 * HID, 50),        # g4: m2d8 [[0,25],[1,5]]
                    (4 * HID, 50),        # g6: m2d8 [[1,5],[0,25]]
                    (5 * HID, 0),         # g7: fp16 m3 [[1,4],[0,125]]
                ]
                BIGPATS = [
                    [[FB, 4], [0, 5], [1, 25]],
                    [[FB, 4], [1, 25], [0, 5]],
                    [[FB, 4], [1, 25], [0, 5]],
                    [[FB, 4], [0, 25], [1, 5]],
                    [[FB, 4], [1, 5], [0, 25]],
                    [[1, 4], [0, 125]],
                ]
                for k, (coff, fboff) in enumerate(BIGS):
                    last = k == len(BIGS) - 1
                    for b in range(2):
                        for t in range(2):
                            lhsT = c8s[t * 64:(t + 1) * 64, coff:coff + HID]
                            if last:
                                rhs = _cap(m3, t * 64, (t + 1) * 64,
                                           po + b * 4, BIGPATS[k])
                            else:
                                rhs = _cap(fb, t * 64, (t + 1) * 64,
                                           (po + b * 4) * FB + fboff,
                                           BIGPATS[k])
                            nc.tensor.matmul(
                                pt[t][:, b * 512:b * 512 + 500], lhsT, rhs,
                                start=False, stop=last,
                                tile_position=(t * 64, 0),
                                skip_group_check=True,
                            )

                # fold for the PREVIOUS window (its relu has completed by now,
                # so the PE never stalls waiting on the Scalar engine)
                if prev is not None:
                    emit_fold(*prev)

                # relu -> fp8 ra (scaled); fold consumes it next window
                ras = []
                for t in range(2):
                    ra = rp.tile([128, 1024], FP8, tag=f"ra{t}", name=f"ra{t}_{w}")
                    nc.scalar.activation(
                        _cap(ra, 0, 128, 0, [[500, 2], [1, 500]]),
                        _cap(pt[t], 0, 128, 0, [[512, 2], [1, 500]]),
                        ACTF.Relu,
                        bias=(b1ra if t == 0 else b2ra)[:, 0:1],
                        scale=scv[:, 3:4])
                    ras.append(ra)
                return (w, ras)

            def emit_fold(w, ras):
                fps = pq.tile([128, 512], F32, tag="ps", name=f"fps_{w}")
                for t in range(2):
                    nc.tensor.matmul(
                        _cap(fps, 0, 128, t * 200, [[25, WPAIRS], [1, 25]]),
                        wo8[:, t * 128:(t + 1) * 128],
                        _cap(ras[t], 0, 128, 0, [[125, WPAIRS], [1, 25]]),
                        start=True, stop=False, skip_group_check=True,
                    )
                    for p in range(WPAIRS):
                        nc.tensor.matmul(
                            _cap(fps, 0, 128, t * 200 + p * 25,
                                 [[0, 4], [1, 25]]),
                            wo8[:, t * 128:(t + 1) * 128],
                            _cap(ras[t], 0, 128, p * 125 + 25, [[1, 100]]),
                            start=False, stop=(p == WPAIRS - 1),
                            skip_group_check=True,
                        )
                nc.vector.tensor_reduce(
                    _cap(pre2, 0, 128, w * WPAIRS, [[NPAIR, 2], [1, WPAIRS]]),
                    _cap(fps, 0, 128, 0, [[200, 2], [25, WPAIRS], [1, 25]]),
                    axis=AXIS.X, op=ALU.add,
                )

            with tc.high_priority():
                for b in range(len(BLOCKS)):
                    emit_feats(b)
            prev = None
            for w in range(NWIN):
                prev = emit_window(w, prev)
            emit_fold(*prev)

            # ---- z = relu(pre2 * sc_z + bout) ----
            z1 = cp.tile([OUT_DIM, NPAIR], F32)
            z2 = cp.tile([OUT_DIM, NPAIR], F32)
            nc.scalar.activation(z1[:, :], pre2[:, 0:NPAIR], ACTF.Relu,
                                 bias=bout1[:, 0:1], scale=scv[:, 4:5])
            nc.scalar.activation(z2[:, :], pre2[:, NPAIR:2 * NPAIR], ACTF.Relu,
                                 bias=bout2[:, 0:1], scale=scv[:, 4:5])

            # ---- final fc: out = z @ fcw + fcb ----
            outsb = cp.tile([128, 2 * (N_LOC // 128)], F32)
            pfc = pq.tile([128, 512], F32, tag="ps")
            for ch in range(N_LOC // 128):
                sl = pfc[:, ch * 8:ch * 8 + 2]
                nc.tensor.matmul(sl, z1[:, ch * 128:(ch + 1) * 128],
                                 fcwhi[:, :], start=True, stop=False)
                nc.tensor.matmul(sl, z2[:, ch * 128:(ch + 1) * 128],
                                 fcwlo[:, :], start=False, stop=False)
                nc.tensor.matmul(sl, ones[:, :], fcb[:, :],
                                 start=False, stop=True)
            nc.vector.tensor_copy(
                _cap(outsb, 0, 128, 0, [[2, 2], [1, 2]]),
                _cap(pfc, 0, 128, 0, [[8, 2], [1, 2]]),
            )
            for ch in range(N_LOC // 128):
                nc.sync.dma_start(out_d[ch * 128:(ch + 1) * 128, :],
                                  outsb[:, ch * 2:(ch + 1) * 2])

    nc.compile()
    return nc


def _pow2_below(x):
    return float(2.0 ** np.floor(np.log2(max(x, 1e-30))))


def make_in_maps(inputs):
    x = np.ascontiguousarray(np.asarray(inputs["x"], dtype=np.int32))
    embed = np.asarray(inputs["embed"], dtype=np.float32)
    coefs1 = np.asarray(inputs["coefs1"], dtype=np.float32)
    coefs2 = np.asarray(inputs["coefs2"], dtype=np.float32)
    wout1 = np.asarray(inputs["wout1"], np.float32)
    wout2 = np.asarray(inputs["wout2"], np.float32)

    e = np.maximum(embed, 0.0)
    emax = float(e.max()) + 1e-12
    # centered-basis features are all bounded by ~emax^3 -> one fp8 scale
    alpha = _pow2_below(200.0 / (emax ** 3))
    # embed pre-scale: t38 = (esc*dl)^3 must stay inside fp8 range
    esc = _pow2_below(200.0 ** (1.0 / 3.0) / emax)
    at3 = esc ** 3
    ra_scale = 32.0
    ws = 1.0

    def stk2(c1, c2, beta):
        m = np.zeros((128, HID), np.float32)
        m[0:64] = c1 * beta
        m[64:128] = c2 * beta
        return m

    def cs_transform(coefs):
        C = [coefs[:, :, g] for g in range(8)]
        return [
            C[0], C[0] + 5 * C[1], C[0] + 5 * C[2], C[0] + 5 * C[3],
            C[0] + 5 * C[1] + 5 * C[2] + 25 * C[4],
            C[0] + 5 * C[1] + 5 * C[3] + 25 * C[5],
            C[0] + 5 * C[2] + 5 * C[3] + 25 * C[6],
            C[0] + 5 * (C[1] + C[2] + C[3])
            + 25 * (C[4] + C[5] + C[6]) + 125 * C[7],
        ]

    Cp1 = cs_transform(coefs1)
    Cp2 = cs_transform(coefs2)

    # fp8 stationary for the (g0,g2) DoubleRow stream: [C'0*s/at3 | C'2*s/a]
    c02 = np.zeros((128, 2 * HID), np.float32)
    c02[:, 0:HID] = stk2(Cp1[0], Cp2[0], SIGMA / at3)
    c02[:, HID:] = stk2(Cp1[2], Cp2[2], SIGMA / alpha)

    # fp16 stationaries, col blocks in stream order [g1 g3 g5 g4 g6 g7]
    c8s = np.zeros((128, 6 * HID), np.float32)
    for bi, g in enumerate([1, 3, 5, 4, 6, 7]):
        c8s[:, bi * HID:(bi + 1) * HID] = stk2(Cp1[g], Cp2[g], SIGMA / alpha)

    wo8 = np.zeros((HID, 2 * OUT_DIM), np.float32)
    wo8[:, 0:OUT_DIM] = wout1 * ws
    wo8[:, OUT_DIM:] = wout2 * ws

    # e_sb carries esc*e; the chain: Sa = S_sb*scv0 (= 5*esc*m*scv0),
    # SU8 = u*Sa = esc^2*dd*Sa -> alpha*m*dd needs scv0 = alpha/(5*esc^3);
    # m2d8 = dl*S2*scv1 = esc^3*25*m^2*dl*scv1 -> scv1 = alpha/(25*esc^3);
    # S3a = S2*Sa = 25*alpha*m^3 -> scv2 = 1/25.
    scv = np.zeros((128, 8), np.float32)
    scv[:, 0] = alpha / (5.0 * at3)
    scv[:, 1] = alpha / (25.0 * at3)
    scv[:, 2] = 0.04
    scv[:, 3] = ra_scale / SIGMA
    scv[:, 4] = 1.0 / (125.0 * ra_scale * ws)

    common = {
        "embed": np.ascontiguousarray(embed * esc),
        "c02": c02,
        "c8s": c8s,
        "wo8": wo8,
        "b1ra": (np.asarray(inputs["bias1"], np.float32) * ra_scale
                 ).reshape(HID, 1).copy(),
        "b2ra": (np.asarray(inputs["bias2"], np.float32) * ra_scale
                 ).reshape(HID, 1).copy(),
        "bout1": np.asarray(inputs["bout1"], np.float32).reshape(OUT_DIM, 1).copy(),
        "bout2": np.asarray(inputs["bout2"], np.float32).reshape(OUT_DIM, 1).copy(),
        "scv": scv,
        "fcwhi": np.ascontiguousarray(np.asarray(inputs["fcw"], np.float32)[0:OUT_DIM]),
        "fcwlo": np.ascontiguousarray(np.asarray(inputs["fcw"], np.float32)[OUT_DIM:]),
        "fcb": np.asarray(inputs["fcb"], np.float32).reshape(1, 2).copy(),
        "ones": np.ones((1, 128), np.float32),
        "iota": np.arange(128, dtype=np.float32).reshape(128, 1).copy(),
    }
    in_maps = []
    for c in range(N_CORES):
        m = dict(common)
        m["x"] = x[c * N_LOC:(c + 1) * N_LOC].reshape(1, N_LOC * 2 * TEAM).copy()
        in_maps.append(m)
    return in_maps


_NC = None


def kernel(**inputs):
    global _NC
    if _NC is None:
        _NC = build_nc()
    in_maps = make_in_maps(inputs)
    res = run_bass_kernel_spmd(_NC, in_maps, core_ids=list(range(N_CORES)))
    return np.concatenate([r["out"] for r in res.results], axis=0)


if __name__ == "__main__":
    nc = build_nc()
    print("built ok")
